# revision 14
# baseline (speedup 1.0000x reference)
"""Trainium2 Bass kernel for the BinaryMechanismSSM problem.

Full inputs in, full outputs out. Batch (128) sharded 8 ways (16 rows/core).

Per core:
  Phase 1: projections bx{0,1} = x @ B{0,1}^T + b (fp16 matmuls, fp32 psum),
           g = sigmoid(x @ G^T + b), 1-g via DVE. Each 512-token tile's
           outputs are collected in wide SBUF staging tiles and stored with
           3 DMAs into group-pair-major DRAM planes (dma_start issue cost
           on the sync engine is ~640ns each, so DMA count is minimized).
  Phase 2: T sequential steps. State held fp16 as slices of an 8-step
           rolling buffer; per step, per half h: one fp16 identity matmul
           injects bx into a [128, NREC*32] PSUM half (start=True), 16 fp16
           A-matmuls accumulate (contraction chunk k reads the previous
           state's half k//2, so matmuls gated on the early half start
           before the late half lands). Tail per half: ACT tanh -> DVE
           pair-add -> (0.5*w)*g -> + (1-g)*s_prev (m2 on GpSimd).
           State is stored to DRAM once per 8 steps.
"""
import numpy as np

B_FULL = 128
T_FULL = 1024
I_DIM = 256
S_DIM = 512
N_CORES = 8
B_LOC = B_FULL // N_CORES  # 16

_cache = {}


def _build(alpha: float, z: int, T: int):
    import concourse.bass as bass
    from concourse import bacc
    import concourse.mybir as mybir
    from concourse.tile import TileContext

    dt = mybir.dt
    AF = mybir.ActivationFunctionType
    ALU = mybir.AluOpType

    TOK = T * B_LOC          # tokens per core
    NTT = TOK // 512         # phase-1 token tiles == group pairs (32 steps)
    NMAT = 3 if z != 0 else 2
    NREC = 2 if z != 0 else 1
    HW = NREC * 32           # psum half width
    WB = NREC * 64           # bx cols per step
    EQ = (z != 0 and abs(alpha - 0.5) < 1e-12)

    nc = bacc.Bacc("TRN2", target_bir_lowering=False, debug=False,
                   num_devices=N_CORES)

    xT_d = nc.declare_dram_parameter("xT", [2, 128, TOK], dt.float16, isOutput=False)
    pw_d = nc.declare_dram_parameter("pw", [128, NMAT * 2 * 4 * 128], dt.float16, isOutput=False)
    bias_d = nc.declare_dram_parameter("bias", [128, 4 * NMAT], dt.float32, isOutput=False)
    aw_d = nc.declare_dram_parameter("aw", [128, NREC * 16 * 128], dt.float16, isOutput=False)
    s0_d = nc.declare_dram_parameter("s0T", [128, 64], dt.float16, isOutput=False)
    iden_d = nc.declare_dram_parameter("iden", [128, 128], dt.float16, isOutput=False)
    stg_d = nc.declare_dram_parameter("stg", [128, T * 64], dt.float16, isOutput=True)

    with TileContext(nc) as tc:
      with tc.tile_pool(name="dram", bufs=1, space="DRAM") as dpool:
        # group-pair-major planes: one contiguous slab per 32 steps
        bxp = dpool.tile([128, NTT * NREC * 4 * 512], dt.float16, tag="bxp",
                         name="bxp")
        hgp = dpool.tile([128, NTT * 4 * 512], dt.float16, tag="hgp",
                         name="hgp")
        gmp = dpool.tile([128, NTT * 4 * 512], dt.float16, tag="gmp",
                         name="gmp")

        # ---------------- Phase 1: projections ----------------
        with (
            tc.tile_pool(name="p1w", bufs=1) as p1w,
            tc.tile_pool(name="p1x", bufs=3) as p1x,
            tc.tile_pool(name="p1o", bufs=2) as p1o,
            tc.tile_pool(name="p1ps", bufs=6, space="PSUM") as p1ps,
        ):
            pw = p1w.tile([128, NMAT * 2 * 4 * 128], dt.float16)
            nc.sync.dma_start(pw[:], pw_d[:])
            bias = p1w.tile([128, 4 * NMAT], dt.float32)
            nc.sync.dma_start(bias[:], bias_d[:])

            for tt in range(NTT):
                xt = p1x.tile([128, 2 * 512], dt.float16, tag="xt")
                for i in range(2):
                    nc.sync.dma_start(xt[:, i * 512:(i + 1) * 512],
                                      xT_d[i, :, tt * 512:(tt + 1) * 512])
                obx = p1o.tile([128, NREC * 4 * 512], dt.float16, tag="obx")
                ohg = p1o.tile([128, 4 * 512], dt.float16, tag="ohg")
                ogm = p1o.tile([128, 4 * 512], dt.float16, tag="ogm")
                for mat in range(NMAT):
                    for c in range(4):
                        ps = p1ps.tile([128, 512], dt.float32, tag="pps")
                        for i in range(2):
                            blk = ((mat * 2 + i) * 4 + c) * 128
                            nc.tensor.matmul(
                                ps[:], pw[:, blk:blk + 128],
                                xt[:, i * 512:(i + 1) * 512],
                                start=(i == 0), stop=(i == 1))
                        bj = bias[:, mat * 4 + c:mat * 4 + c + 1]
                        if mat < NREC:
                            nc.vector.tensor_scalar(
                                obx[:, (mat * 4 + c) * 512:
                                    (mat * 4 + c + 1) * 512],
                                ps[:], bj, None, ALU.add)
                        else:
                            sg = ohg[:, c * 512:(c + 1) * 512]
                            nc.scalar.activation(sg, ps[:], AF.Sigmoid,
                                                 bias=bj, scale=1.0)
                            nc.vector.tensor_scalar(
                                ogm[:, c * 512:(c + 1) * 512],
                                sg, -1.0, 1.0, ALU.mult, ALU.add)
                W1 = NREC * 4 * 512
                nc.sync.dma_start(bxp[:, tt * W1:(tt + 1) * W1], obx[:])
                nc.sync.dma_start(hgp[:, tt * 2048:(tt + 1) * 2048], ohg[:])
                nc.sync.dma_start(gmp[:, tt * 2048:(tt + 1) * 2048], ogm[:])

        # ---------------- Phase 2: recurrence ----------------
        with (
            tc.tile_pool(name="p2w", bufs=1) as p2w,
            tc.tile_pool(name="p2g", bufs=2) as p2g,
            tc.tile_pool(name="p2s", bufs=3) as p2s,
            tc.tile_pool(name="p2c", bufs=6) as p2c,
            tc.tile_pool(name="p2ps", bufs=1, space="PSUM") as p2ps,
        ):
            aw = p2w.tile([128, NREC * 16 * 128], dt.float16)
            nc.sync.dma_start(aw[:], aw_d[:])
            iden = p2w.tile([128, 128], dt.float16)
            nc.sync.dma_start(iden[:], iden_d[:])

            s_init = p2w.tile([128, 64], dt.float16)
            nc.sync.dma_start(s_init[:], s0_d[:])
            sth = [s_init[:, 0:32], s_init[:, 32:64]]

            def ablk(m, c, k):
                return ((m * 4 + c) * 4 + k) * 128

            def emit_gp_loads(gp):
                W1 = NREC * 4 * 512
                bxg = p2g.tile([128, W1], dt.float16, tag="bxg", name="bxg")
                nc.sync.dma_start(bxg[:], bxp[:, gp * W1:(gp + 1) * W1])
                hgg = p2g.tile([128, 2048], dt.float16, tag="hgg", name="hgg")
                nc.sync.dma_start(hgg[:], hgp[:, gp * 2048:(gp + 1) * 2048])
                gmg = p2g.tile([128, 2048], dt.float16, tag="gmg", name="gmg")
                nc.sync.dma_start(gmg[:], gmp[:, gp * 2048:(gp + 1) * 2048])
                return bxg, hgg, gmg

            cur = emit_gp_loads(0)
            nxt = None
            stag = None
            for gp in range(NTT):
                bxg, hgg, gmg = cur
                bxg_v = bxg[:].rearrange("p (m c t b) -> p m c t b",
                                         m=NREC, c=4, t=32, b=16)
                hgg_v = hgg[:].rearrange("p (c t b) -> p c t b",
                                         c=4, t=32, b=16)
                gmg_v = gmg[:].rearrange("p (c t b) -> p c t b",
                                         c=4, t=32, b=16)
                for tl in range(32):
                    t = gp * 32 + tl
                    if tl == 4 and gp + 1 < NTT:
                        nxt = emit_gp_loads(gp + 1)
                    if t % 8 == 0:
                        stag = p2s.tile([128, 512], dt.float16, tag="stbuf",
                                        name="stbuf")
                    off = (t % 8) * 64
                    psc = [p2ps.tile([128, HW], dt.float32,
                                     tag=f"psc{h}{t % 4}",
                                     name=f"psc{h}{t % 4}")
                           for h in range(2)]
                    for h in range(2):
                        nc.tensor.matmul(
                            psc[h][:], iden[:],
                            bxg_v[:, :, 2 * h:2 * h + 2, tl, :],
                            start=True, stop=False)

                    def amms(hreg, ks, stop_k):
                        for c in (2 * hreg, 2 * hreg + 1):
                            for m in range(NREC):
                                for k in ks:
                                    nc.tensor.matmul(
                                        psc[hreg][:, m * 32 + (c % 2) * 16:
                                                  m * 32 + (c % 2) * 16 + 16],
                                        aw[:, ablk(m, c, k):ablk(m, c, k) + 128],
                                        sth[k // 2][:, (k % 2) * 16:
                                                    (k % 2) * 16 + 16],
                                        start=False, stop=(k == stop_k))
                    new_sth = [None, None]
                    for h in (0, 1):
                        amms(h, (0, 1), -1)   # this half, sth0-gated
                        amms(h, (2, 3), 3)    # finishers (sth1-gated)
                        ft = p2c.tile([128, HW], dt.float16, tag=f"ft{h}")
                        nc.scalar.activation(ft[:], psc[h][:], AF.Tanh)
                        hgs = hgg_v[:, 2 * h:2 * h + 2, tl, :]
                        gms = gmg_v[:, 2 * h:2 * h + 2, tl, :]
                        m2h = p2c.tile([128, 32], dt.float16, tag=f"m2{h}")
                        nc.gpsimd.tensor_tensor(m2h[:], sth[h], gms, ALU.mult)
                        s_new = stag[:, off + h * 32:off + (h + 1) * 32]
                        if NREC == 2 and EQ:
                            w = p2c.tile([128, 32], dt.float16, tag=f"w{h}")
                            nc.vector.tensor_tensor(
                                w[:], ft[:, 0:32], ft[:, 32:64], ALU.add)
                            q = p2c.tile([128, 32], dt.float16, tag=f"q{h}")
                            nc.vector.scalar_tensor_tensor(
                                q[:], w[:], 0.5, hgs, ALU.mult, ALU.mult)
                        elif NREC == 2:
                            mc = p2c.tile([128, 64], dt.float16, tag=f"mc{h}")
                            nc.vector.scalar_tensor_tensor(
                                mc[:, 0:32], ft[:, 0:32], 1.0 - alpha,
                                hgs, ALU.mult, ALU.mult)
                            nc.vector.scalar_tensor_tensor(
                                mc[:, 32:64], ft[:, 32:64], alpha,
                                hgs, ALU.mult, ALU.mult)
                            q = p2c.tile([128, 32], dt.float16, tag=f"q{h}")
                            nc.vector.tensor_tensor(
                                q[:], mc[:, 0:32], mc[:, 32:64], ALU.add)
                        else:
                            q = p2c.tile([128, 32], dt.float16, tag=f"q{h}")
                            nc.vector.scalar_tensor_tensor(
                                q[:], ft[:], 1.0, hgs, ALU.mult, ALU.mult)
                        nc.vector.tensor_tensor(s_new, q[:], m2h[:], ALU.add)
                        new_sth[h] = s_new
                    sth = new_sth
                    if t % 8 == 7:
                        nc.sync.dma_start(
                            stg_d[:, (t - 7) * 64:(t + 1) * 64], stag[:])
                cur = nxt

    nc.compile()
    return nc


def _pack_lhsT_blocks(Wm, kdim, mdim, dtype):
    """Wm: [mdim*128, kdim*128]; returns [128, kdim*mdim*128] with block
    (k, j) at cols (k*mdim+j)*128 equal to Wm[j-chunk, k-chunk].T."""
    out = np.zeros((128, kdim * mdim * 128), dtype=dtype)
    for k in range(kdim):
        for j in range(mdim):
            blk = Wm[j * 128:(j + 1) * 128, k * 128:(k + 1) * 128].T
            out[:, (k * mdim + j) * 128:(k * mdim + j + 1) * 128] = blk
    return np.ascontiguousarray(out)


def kernel(x_seq, s0, A0_w, B0_w, B0_b, A1_w, B1_w, B1_b, gate_w, gate_b,
           alpha, z, _T=None, _trace=False):
    from concourse.bass_utils import run_bass_kernel_spmd

    T = int(_T or T_FULL)
    alpha_f = float(np.asarray(alpha))
    z_i = int(np.asarray(z))

    key = (alpha_f, z_i, T)
    if key not in _cache:
        _cache[key] = _build(alpha_f, z_i, T)
    nc = _cache[key]

    NMAT = 3 if z_i != 0 else 2
    NREC = 2 if z_i != 0 else 1

    x_seq = np.asarray(x_seq, dtype=np.float32)
    s0 = np.asarray(s0, dtype=np.float32)

    # ---- shared (replicated) weight packing ----
    mats = ([np.asarray(B0_w), np.asarray(B1_w), np.asarray(gate_w)]
            if z_i != 0 else [np.asarray(B0_w), np.asarray(gate_w)])
    biases = ([np.asarray(B0_b), np.asarray(B1_b), np.asarray(gate_b)]
              if z_i != 0 else [np.asarray(B0_b), np.asarray(gate_b)])
    # phase-1 lhsT blocks per matrix: (mat, i, c) at col ((mat*2+i)*4+c)*128
    pw = np.concatenate(
        [_pack_lhsT_blocks(Wm.astype(np.float32), 2, 4, np.float32)
         for Wm in mats], axis=1).astype(np.float16)
    pw = np.ascontiguousarray(pw)

    bias = np.zeros((128, 4 * NMAT), np.float32)
    for mi, bvec in enumerate(biases):
        bias[:, mi * 4:(mi + 1) * 4] = bvec.astype(np.float32).reshape(4, 128).T

    recs = [np.asarray(A0_w)] if z_i == 0 else [np.asarray(A0_w), np.asarray(A1_w)]
    # phase-2 lhsT block (m, c, k) at col ((m*4+c)*4+k)*128 = A_m[c128, k128].T
    aw_list = []
    for A in recs:
        Af = A.astype(np.float32)
        blocks = np.zeros((128, 16 * 128), np.float32)
        for c in range(4):
            for k in range(4):
                blocks[:, (c * 4 + k) * 128:(c * 4 + k + 1) * 128] = \
                    Af[c * 128:(c + 1) * 128, k * 128:(k + 1) * 128].T
        aw_list.append(blocks)
    aw = np.ascontiguousarray(np.concatenate(aw_list, axis=1).astype(np.float16))

    IDEN = np.ascontiguousarray(np.eye(128, dtype=np.float16))

    # ---- per-core inputs ----
    in_maps = []
    for cix in range(N_CORES):
        bc = cix * B_LOC
        xc = x_seq[bc:bc + B_LOC, :T]                       # [16, T, 256]
        xT = np.ascontiguousarray(
            xc.transpose(2, 1, 0).reshape(2, 128, T * B_LOC)).astype(np.float16)
        s0c = s0[bc:bc + B_LOC]                             # [16, 512]
        s0T = np.ascontiguousarray(
            s0c.T.reshape(4, 128, B_LOC).transpose(1, 0, 2).reshape(128, 64)
        ).astype(np.float16)
        in_maps.append({
            "xT": xT, "pw": pw, "bias": bias, "aw": aw, "s0T": s0T,
            "iden": IDEN,
        })

    res = run_bass_kernel_spmd(nc, in_maps, list(range(N_CORES)), trace=_trace)
    if _trace:
        kernel._last_res = res

    out = np.empty((B_FULL, T + 1, S_DIM), np.float32)
    for cix in range(N_CORES):
        bc = cix * B_LOC
        stg = np.asarray(res.results[cix]["stg"], dtype=np.float32)  # [128, T*64]
        out[bc:bc + B_LOC, 0] = s0[bc:bc + B_LOC]
        out[bc:bc + B_LOC, 1:] = (
            stg.reshape(128, T, 4, B_LOC).transpose(3, 1, 2, 0)
            .reshape(B_LOC, T, S_DIM))
    return out


# revision 15
# speedup vs baseline: 1.1895x; 1.1895x over previous
"""Trainium2 Bass kernel for the BinaryMechanismSSM problem.

Full inputs in, full outputs out. Batch (128) sharded 8 ways (16 rows/core).

Per core:
  Phase 1: projections bx{0,1} = x @ B{0,1}^T + b (fp16 matmuls, fp32 psum),
           g = sigmoid(x @ G^T + b), 1-g via DVE. Each 512-token tile's
           outputs are collected in wide SBUF staging tiles and stored with
           3 DMAs into group-pair-major DRAM planes (dma_start issue cost
           on the sync engine is ~640ns each, so DMA count is minimized).
  Phase 2: T sequential steps. State held fp16 as slices of an 8-step
           rolling buffer; per step, per half h: one fp16 identity matmul
           injects bx into a [128, NREC*32] PSUM half (start=True), 16 fp16
           A-matmuls accumulate (contraction chunk k reads the previous
           state's half k//2, so matmuls gated on the early half start
           before the late half lands). Tail per half: ACT tanh -> DVE
           pair-add -> (0.5*w)*g -> + (1-g)*s_prev (m2 on GpSimd).
           State is stored to DRAM once per 8 steps.
"""
import numpy as np

B_FULL = 128
T_FULL = 1024
I_DIM = 256
S_DIM = 512
N_CORES = 8
B_LOC = B_FULL // N_CORES  # 16

_cache = {}


def _build(alpha: float, z: int, T: int):
    import concourse.bass as bass
    from concourse import bacc
    import concourse.mybir as mybir
    from concourse.tile import TileContext

    dt = mybir.dt
    AF = mybir.ActivationFunctionType
    ALU = mybir.AluOpType

    TOK = T * B_LOC          # tokens per core
    NTT = TOK // 512         # phase-1 token tiles == group pairs (32 steps)
    NMAT = 3 if z != 0 else 2
    NREC = 2 if z != 0 else 1
    HW = NREC * 32           # psum half width
    WB = NREC * 64           # bx cols per step
    EQ = (z != 0 and abs(alpha - 0.5) < 1e-12)

    nc = bacc.Bacc("TRN2", target_bir_lowering=False, debug=False,
                   num_devices=N_CORES)

    xT_d = nc.declare_dram_parameter("xT", [2, 128, TOK], dt.float16, isOutput=False)
    pw_d = nc.declare_dram_parameter("pw", [128, NMAT * 2 * 4 * 128], dt.float16, isOutput=False)
    bias_d = nc.declare_dram_parameter("bias", [128, 4 * NMAT], dt.float32, isOutput=False)
    aw_d = nc.declare_dram_parameter("aw", [128, NREC * 16 * 128], dt.float16, isOutput=False)
    s0_d = nc.declare_dram_parameter("s0T", [128, 64], dt.float16, isOutput=False)
    iden_d = nc.declare_dram_parameter("iden", [128, 128], dt.float16, isOutput=False)
    stg_d = nc.declare_dram_parameter("stg", [128, T * 64], dt.float16, isOutput=True)

    with TileContext(nc) as tc:
      with tc.tile_pool(name="dram", bufs=1, space="DRAM") as dpool:
        # group-pair-major planes: one contiguous slab per 32 steps
        bxp = dpool.tile([128, NTT * NREC * 4 * 512], dt.float16, tag="bxp",
                         name="bxp")
        hgp = dpool.tile([128, NTT * 4 * 512], dt.float16, tag="hgp",
                         name="hgp")
        gmp = dpool.tile([128, NTT * 4 * 512], dt.float16, tag="gmp",
                         name="gmp")

        # ---------------- Phase 1: projections ----------------
        with (
            tc.tile_pool(name="p1w", bufs=1) as p1w,
            tc.tile_pool(name="p1x", bufs=3) as p1x,
            tc.tile_pool(name="p1o", bufs=2) as p1o,
            tc.tile_pool(name="p1ps", bufs=6, space="PSUM") as p1ps,
        ):
            pw = p1w.tile([128, NMAT * 2 * 4 * 128], dt.float16)
            nc.sync.dma_start(pw[:], pw_d[:])
            bias = p1w.tile([128, 4 * NMAT], dt.float32)
            nc.sync.dma_start(bias[:], bias_d[:])

            for tt in range(NTT):
                xt = p1x.tile([128, 2 * 512], dt.float16, tag="xt")
                for i in range(2):
                    nc.sync.dma_start(xt[:, i * 512:(i + 1) * 512],
                                      xT_d[i, :, tt * 512:(tt + 1) * 512])
                obx = p1o.tile([128, NREC * 4 * 512], dt.float16, tag="obx")
                ohg = p1o.tile([128, 4 * 512], dt.float16, tag="ohg")
                ogm = p1o.tile([128, 4 * 512], dt.float16, tag="ogm")
                for mat in range(NMAT):
                    for c in range(4):
                        ps = p1ps.tile([128, 512], dt.float32, tag="pps")
                        for i in range(2):
                            blk = ((mat * 2 + i) * 4 + c) * 128
                            nc.tensor.matmul(
                                ps[:], pw[:, blk:blk + 128],
                                xt[:, i * 512:(i + 1) * 512],
                                start=(i == 0), stop=(i == 1))
                        bj = bias[:, mat * 4 + c:mat * 4 + c + 1]
                        if mat < NREC:
                            nc.vector.tensor_scalar(
                                obx[:, (mat * 4 + c) * 512:
                                    (mat * 4 + c + 1) * 512],
                                ps[:], bj, None, ALU.add)
                        else:
                            sg = ohg[:, c * 512:(c + 1) * 512]
                            nc.scalar.activation(sg, ps[:], AF.Sigmoid,
                                                 bias=bj, scale=1.0)
                            nc.vector.tensor_scalar(
                                ogm[:, c * 512:(c + 1) * 512],
                                sg, -1.0, 1.0, ALU.mult, ALU.add)
                W1 = NREC * 4 * 512
                nc.sync.dma_start(bxp[:, tt * W1:(tt + 1) * W1], obx[:])
                nc.sync.dma_start(hgp[:, tt * 2048:(tt + 1) * 2048], ohg[:])
                nc.sync.dma_start(gmp[:, tt * 2048:(tt + 1) * 2048], ogm[:])

        # ---------------- Phase 2: recurrence ----------------
        with (
            tc.tile_pool(name="p2w", bufs=1) as p2w,
            tc.tile_pool(name="p2g", bufs=2) as p2g,
            tc.tile_pool(name="p2s", bufs=3) as p2s,
            tc.tile_pool(name="p2c", bufs=6) as p2c,
            tc.tile_pool(name="p2ps", bufs=1, space="PSUM") as p2ps,
        ):
            aw = p2w.tile([128, NREC * 16 * 128], dt.float16)
            nc.sync.dma_start(aw[:], aw_d[:])
            iden = p2w.tile([128, 128], dt.float16)
            nc.sync.dma_start(iden[:], iden_d[:])

            s_init = p2w.tile([128, 64], dt.float16)
            nc.sync.dma_start(s_init[:], s0_d[:])
            sth = [s_init[:, 0:32], s_init[:, 32:64]]

            def ablk(m, c, k):
                return ((m * 4 + c) * 4 + k) * 128

            def emit_gp_loads(gp):
                W1 = NREC * 4 * 512
                bxg = p2g.tile([128, W1], dt.float16, tag="bxg", name="bxg")
                nc.sync.dma_start(bxg[:], bxp[:, gp * W1:(gp + 1) * W1])
                hgg = p2g.tile([128, 2048], dt.float16, tag="hgg", name="hgg")
                nc.sync.dma_start(hgg[:], hgp[:, gp * 2048:(gp + 1) * 2048])
                gmg = p2g.tile([128, 2048], dt.float16, tag="gmg", name="gmg")
                nc.sync.dma_start(gmg[:], gmp[:, gp * 2048:(gp + 1) * 2048])
                return bxg, hgg, gmg

            cur = emit_gp_loads(0)
            nxt = None
            stag = None
            for gp in range(NTT):
                bxg, hgg, gmg = cur
                bxg_v = bxg[:].rearrange("p (m c t b) -> p m c t b",
                                         m=NREC, c=4, t=32, b=16)
                hgg_v = hgg[:].rearrange("p (c t b) -> p c t b",
                                         c=4, t=32, b=16)
                gmg_v = gmg[:].rearrange("p (c t b) -> p c t b",
                                         c=4, t=32, b=16)
                for tl in range(32):
                    t = gp * 32 + tl
                    if tl == 4 and gp + 1 < NTT:
                        nxt = emit_gp_loads(gp + 1)
                    if t % 8 == 0:
                        stag = p2s.tile([128, 512], dt.float16, tag="stbuf",
                                        name="stbuf")
                    off = (t % 8) * 64
                    psc = [p2ps.tile([128, HW], dt.float32,
                                     tag=f"psc{h}{t % 4}",
                                     name=f"psc{h}{t % 4}")
                           for h in range(2)]
                    for h in range(2):
                        nc.tensor.matmul(
                            psc[h][:], iden[:],
                            bxg_v[:, :, 2 * h:2 * h + 2, tl, :],
                            start=True, stop=False)

                    def amms(hreg, ks, stop_k):
                        for c in (2 * hreg, 2 * hreg + 1):
                            for m in range(NREC):
                                for k in ks:
                                    nc.tensor.matmul(
                                        psc[hreg][:, m * 32 + (c % 2) * 16:
                                                  m * 32 + (c % 2) * 16 + 16],
                                        aw[:, ablk(m, c, k):ablk(m, c, k) + 128],
                                        sth[k // 2][:, (k % 2) * 16:
                                                    (k % 2) * 16 + 16],
                                        start=False, stop=(k == stop_k))
                    amms(0, (0, 1), -1)   # h0 regions, early (sth0-gated)
                    amms(0, (2, 3), 3)    # h0 finishers (sth1-gated)
                    amms(1, (0, 1), -1)   # h1 regions (sth0-gated)
                    amms(1, (2, 3), 3)    # h1 finishers

                    new_sth = [None, None]
                    for h in (0, 1):
                        ft = p2c.tile([128, HW], dt.float16, tag=f"ft{h}")
                        nc.scalar.activation(ft[:], psc[h][:], AF.Tanh)
                        hgs = hgg_v[:, 2 * h:2 * h + 2, tl, :]
                        gms = gmg_v[:, 2 * h:2 * h + 2, tl, :]
                        m2h = p2c.tile([128, 32], dt.float16, tag=f"m2{h}")
                        nc.gpsimd.tensor_tensor(m2h[:], sth[h], gms, ALU.mult)
                        s_new = stag[:, off + h * 32:off + (h + 1) * 32]
                        if NREC == 2 and EQ:
                            w = p2c.tile([128, 32], dt.float16, tag=f"w{h}")
                            nc.vector.tensor_tensor(
                                w[:], ft[:, 0:32], ft[:, 32:64], ALU.add)
                            q = p2c.tile([128, 32], dt.float16, tag=f"q{h}")
                            nc.vector.scalar_tensor_tensor(
                                q[:], w[:], 0.5, hgs, ALU.mult, ALU.mult)
                        elif NREC == 2:
                            mc = p2c.tile([128, 64], dt.float16, tag=f"mc{h}")
                            nc.vector.scalar_tensor_tensor(
                                mc[:, 0:32], ft[:, 0:32], 1.0 - alpha,
                                hgs, ALU.mult, ALU.mult)
                            nc.vector.scalar_tensor_tensor(
                                mc[:, 32:64], ft[:, 32:64], alpha,
                                hgs, ALU.mult, ALU.mult)
                            q = p2c.tile([128, 32], dt.float16, tag=f"q{h}")
                            nc.vector.tensor_tensor(
                                q[:], mc[:, 0:32], mc[:, 32:64], ALU.add)
                        else:
                            q = p2c.tile([128, 32], dt.float16, tag=f"q{h}")
                            nc.vector.scalar_tensor_tensor(
                                q[:], ft[:], 1.0, hgs, ALU.mult, ALU.mult)
                        nc.vector.tensor_tensor(s_new, q[:], m2h[:], ALU.add)
                        new_sth[h] = s_new
                    sth = new_sth
                    if t % 8 == 7:
                        nc.sync.dma_start(
                            stg_d[:, (t - 7) * 64:(t + 1) * 64], stag[:])
                cur = nxt

    nc.compile()
    return nc


def _pack_lhsT_blocks(Wm, kdim, mdim, dtype):
    """Wm: [mdim*128, kdim*128]; returns [128, kdim*mdim*128] with block
    (k, j) at cols (k*mdim+j)*128 equal to Wm[j-chunk, k-chunk].T."""
    out = np.zeros((128, kdim * mdim * 128), dtype=dtype)
    for k in range(kdim):
        for j in range(mdim):
            blk = Wm[j * 128:(j + 1) * 128, k * 128:(k + 1) * 128].T
            out[:, (k * mdim + j) * 128:(k * mdim + j + 1) * 128] = blk
    return np.ascontiguousarray(out)


def kernel(x_seq, s0, A0_w, B0_w, B0_b, A1_w, B1_w, B1_b, gate_w, gate_b,
           alpha, z, _T=None, _trace=False):
    from concourse.bass_utils import run_bass_kernel_spmd

    T = int(_T or T_FULL)
    alpha_f = float(np.asarray(alpha))
    z_i = int(np.asarray(z))

    key = (alpha_f, z_i, T)
    if key not in _cache:
        _cache[key] = _build(alpha_f, z_i, T)
    nc = _cache[key]

    NMAT = 3 if z_i != 0 else 2
    NREC = 2 if z_i != 0 else 1

    x_seq = np.asarray(x_seq, dtype=np.float32)
    s0 = np.asarray(s0, dtype=np.float32)

    # ---- shared (replicated) weight packing ----
    mats = ([np.asarray(B0_w), np.asarray(B1_w), np.asarray(gate_w)]
            if z_i != 0 else [np.asarray(B0_w), np.asarray(gate_w)])
    biases = ([np.asarray(B0_b), np.asarray(B1_b), np.asarray(gate_b)]
              if z_i != 0 else [np.asarray(B0_b), np.asarray(gate_b)])
    # phase-1 lhsT blocks per matrix: (mat, i, c) at col ((mat*2+i)*4+c)*128
    pw = np.concatenate(
        [_pack_lhsT_blocks(Wm.astype(np.float32), 2, 4, np.float32)
         for Wm in mats], axis=1).astype(np.float16)
    pw = np.ascontiguousarray(pw)

    bias = np.zeros((128, 4 * NMAT), np.float32)
    for mi, bvec in enumerate(biases):
        bias[:, mi * 4:(mi + 1) * 4] = bvec.astype(np.float32).reshape(4, 128).T

    recs = [np.asarray(A0_w)] if z_i == 0 else [np.asarray(A0_w), np.asarray(A1_w)]
    # phase-2 lhsT block (m, c, k) at col ((m*4+c)*4+k)*128 = A_m[c128, k128].T
    aw_list = []
    for A in recs:
        Af = A.astype(np.float32)
        blocks = np.zeros((128, 16 * 128), np.float32)
        for c in range(4):
            for k in range(4):
                blocks[:, (c * 4 + k) * 128:(c * 4 + k + 1) * 128] = \
                    Af[c * 128:(c + 1) * 128, k * 128:(k + 1) * 128].T
        aw_list.append(blocks)
    aw = np.ascontiguousarray(np.concatenate(aw_list, axis=1).astype(np.float16))

    IDEN = np.ascontiguousarray(np.eye(128, dtype=np.float16))

    # ---- per-core inputs ----
    in_maps = []
    for cix in range(N_CORES):
        bc = cix * B_LOC
        xc = x_seq[bc:bc + B_LOC, :T]                       # [16, T, 256]
        xT = np.ascontiguousarray(
            xc.transpose(2, 1, 0).reshape(2, 128, T * B_LOC)).astype(np.float16)
        s0c = s0[bc:bc + B_LOC]                             # [16, 512]
        s0T = np.ascontiguousarray(
            s0c.T.reshape(4, 128, B_LOC).transpose(1, 0, 2).reshape(128, 64)
        ).astype(np.float16)
        in_maps.append({
            "xT": xT, "pw": pw, "bias": bias, "aw": aw, "s0T": s0T,
            "iden": IDEN,
        })

    res = run_bass_kernel_spmd(nc, in_maps, list(range(N_CORES)), trace=_trace)
    if _trace:
        kernel._last_res = res

    out = np.empty((B_FULL, T + 1, S_DIM), np.float32)
    for cix in range(N_CORES):
        bc = cix * B_LOC
        stg = np.asarray(res.results[cix]["stg"], dtype=np.float32)  # [128, T*64]
        out[bc:bc + B_LOC, 0] = s0[bc:bc + B_LOC]
        out[bc:bc + B_LOC, 1:] = (
            stg.reshape(128, T, 4, B_LOC).transpose(3, 1, 2, 0)
            .reshape(B_LOC, T, S_DIM))
    return out


# revision 18
# speedup vs baseline: 1.2437x; 1.0456x over previous
"""Trainium2 Bass kernel for the BinaryMechanismSSM problem.

Full inputs in, full outputs out. Batch (128) sharded 8 ways (16 rows/core).

Per core:
  Phase 1: projections bx{0,1} = x @ B{0,1}^T + b (fp16 matmuls, fp32 psum),
           g = sigmoid(x @ G^T + b), 1-g via DVE. Each 512-token tile's
           outputs are collected in wide SBUF staging tiles and stored with
           3 DMAs into group-pair-major DRAM planes (dma_start issue cost
           on the sync engine is ~640ns each, so DMA count is minimized).
  Phase 2: T sequential steps. State held fp16 as slices of an 8-step
           rolling buffer; per step, per half h: one fp16 identity matmul
           injects bx into a [128, NREC*32] PSUM half (start=True), 16 fp16
           A-matmuls accumulate (contraction chunk k reads the previous
           state's half k//2, so matmuls gated on the early half start
           before the late half lands). Tail per half: ACT tanh -> DVE
           pair-add -> (0.5*w)*g -> + (1-g)*s_prev (m2 on GpSimd).
           State is stored to DRAM once per 8 steps.
"""
import numpy as np

B_FULL = 128
T_FULL = 1024
I_DIM = 256
S_DIM = 512
N_CORES = 8
B_LOC = B_FULL // N_CORES  # 16

_cache = {}


def _build(alpha: float, z: int, T: int):
    import concourse.bass as bass
    from concourse import bacc
    import concourse.mybir as mybir
    from concourse.tile import TileContext

    dt = mybir.dt
    AF = mybir.ActivationFunctionType
    ALU = mybir.AluOpType

    TOK = T * B_LOC          # tokens per core
    NTT = TOK // 512         # phase-1 token tiles == group pairs (32 steps)
    NMAT = 3 if z != 0 else 2
    NREC = 2 if z != 0 else 1
    HW = NREC * 32           # psum half width
    WB = NREC * 64           # bx cols per step
    EQ = (z != 0 and abs(alpha - 0.5) < 1e-12)

    nc = bacc.Bacc("TRN2", target_bir_lowering=False, debug=False,
                   num_devices=N_CORES)

    xT_d = nc.declare_dram_parameter("xT", [2, 128, TOK], dt.float16, isOutput=False)
    pw_d = nc.declare_dram_parameter("pw", [128, NMAT * 2 * 4 * 128], dt.float16, isOutput=False)
    bias_d = nc.declare_dram_parameter("bias", [128, 4 * NMAT], dt.float32, isOutput=False)
    aw_d = nc.declare_dram_parameter("aw", [128, NREC * 16 * 128], dt.float16, isOutput=False)
    s0_d = nc.declare_dram_parameter("s0T", [128, 64], dt.float16, isOutput=False)
    iden_d = nc.declare_dram_parameter("iden", [128, 128], dt.float16, isOutput=False)
    stg_d = nc.declare_dram_parameter("stg", [128, T * 64], dt.float16, isOutput=True)

    with TileContext(nc) as tc:
      with tc.tile_pool(name="dram", bufs=1, space="DRAM") as dpool:
        # group-pair-major planes: one contiguous slab per 32 steps
        bxp = dpool.tile([128, NTT * NREC * 4 * 512], dt.float16, tag="bxp",
                         name="bxp")
        hgp = dpool.tile([128, NTT * 4 * 512], dt.float16, tag="hgp",
                         name="hgp")
        gmp = dpool.tile([128, NTT * 4 * 512], dt.float16, tag="gmp",
                         name="gmp")

        # ---------------- pools (both phases coexist; phase 1 is pumped
        # into phase 2's per-step idle gaps) ----------------
        with (
            tc.tile_pool(name="p1w", bufs=1) as p1w,
            tc.tile_pool(name="p1x", bufs=3) as p1x,
            tc.tile_pool(name="p1o", bufs=2) as p1o,
            tc.tile_pool(name="p1ps", bufs=2, space="PSUM") as p1ps,
            tc.tile_pool(name="p2w", bufs=1) as p2w,
            tc.tile_pool(name="p2g", bufs=2) as p2g,
            tc.tile_pool(name="p2s", bufs=3) as p2s,
            tc.tile_pool(name="p2c", bufs=6) as p2c,
            tc.tile_pool(name="p2ps", bufs=1, space="PSUM") as p2ps,
        ):
            pw = p1w.tile([128, NMAT * 2 * 4 * 128], dt.float16)
            nc.sync.dma_start(pw[:], pw_d[:])
            bias = p1w.tile([128, 4 * NMAT], dt.float32)
            nc.sync.dma_start(bias[:], bias_d[:])

            def p1_units():
                """Phase-1 work as small bundles; yields between bundles so
                they can be sprinkled across phase-2 steps."""
                W1 = NREC * 4 * 512
                for tt in range(NTT):
                    xt = p1x.tile([128, 2 * 512], dt.float16, tag="xt",
                                  name="xt")
                    for i in range(2):
                        nc.sync.dma_start(xt[:, i * 512:(i + 1) * 512],
                                          xT_d[i, :, tt * 512:(tt + 1) * 512])
                    obx = p1o.tile([128, NREC * 4 * 512], dt.float16,
                                   tag="obx", name="obx")
                    ohg = p1o.tile([128, 4 * 512], dt.float16, tag="ohg",
                                   name="ohg")
                    ogm = p1o.tile([128, 4 * 512], dt.float16, tag="ogm",
                                   name="ogm")
                    for mat in range(NMAT):
                        for c in range(4):
                            ps = p1ps.tile([128, 512], dt.float32, tag="pps",
                                           name="pps")
                            for i in range(2):
                                blk = ((mat * 2 + i) * 4 + c) * 128
                                nc.tensor.matmul(
                                    ps[:], pw[:, blk:blk + 128],
                                    xt[:, i * 512:(i + 1) * 512],
                                    start=(i == 0), stop=(i == 1))
                            bj = bias[:, mat * 4 + c:mat * 4 + c + 1]
                            if mat < NREC:
                                o = obx[:, (mat * 4 + c) * 512:
                                        (mat * 4 + c + 1) * 512]
                                # halves, alternating DVE/ACT: small quanta
                                # that don't block the recurrence chain
                                for ih in range(2):
                                    osl = o[:, ih * 256:(ih + 1) * 256]
                                    psl = ps[:, ih * 256:(ih + 1) * 256]
                                    if (mat * 4 + c) % 2 == 0:
                                        nc.vector.tensor_scalar(
                                            osl, psl, bj, None, ALU.add)
                                    else:
                                        nc.scalar.activation(
                                            osl, psl, AF.Identity,
                                            bias=bj, scale=1.0)
                            else:
                                sg = ohg[:, c * 512:(c + 1) * 512]
                                for ih in range(2):
                                    nc.scalar.activation(
                                        sg[:, ih * 256:(ih + 1) * 256],
                                        ps[:, ih * 256:(ih + 1) * 256],
                                        AF.Sigmoid, bias=bj, scale=1.0)
                                nc.gpsimd.tensor_scalar(
                                    ogm[:, c * 512:(c + 1) * 512],
                                    sg, -1.0, 1.0, ALU.mult, ALU.add)
                            yield
                    nc.sync.dma_start(bxp[:, tt * W1:(tt + 1) * W1], obx[:])
                    nc.sync.dma_start(hgp[:, tt * 2048:(tt + 1) * 2048],
                                      ohg[:])
                    nc.sync.dma_start(gmp[:, tt * 2048:(tt + 1) * 2048],
                                      ogm[:])
                    yield

            UNITS_PER_TILE = 4 * NMAT + 1
            LEAD_TILES = min(4, NTT)
            p1gen = p1_units()
            for _ in range(LEAD_TILES * UNITS_PER_TILE):
                next(p1gen, None)
            aw = p2w.tile([128, NREC * 16 * 128], dt.float16)
            nc.sync.dma_start(aw[:], aw_d[:])
            iden = p2w.tile([128, 128], dt.float16)
            nc.sync.dma_start(iden[:], iden_d[:])

            s_init = p2w.tile([128, 64], dt.float16)
            nc.sync.dma_start(s_init[:], s0_d[:])
            sth = [s_init[:, 0:32], s_init[:, 32:64]]

            def ablk(m, c, k):
                return ((m * 4 + c) * 4 + k) * 128

            def emit_gp_loads(gp):
                W1 = NREC * 4 * 512
                bxg = p2g.tile([128, W1], dt.float16, tag="bxg", name="bxg")
                nc.sync.dma_start(bxg[:], bxp[:, gp * W1:(gp + 1) * W1])
                hgg = p2g.tile([128, 2048], dt.float16, tag="hgg", name="hgg")
                nc.sync.dma_start(hgg[:], hgp[:, gp * 2048:(gp + 1) * 2048])
                gmg = p2g.tile([128, 2048], dt.float16, tag="gmg", name="gmg")
                nc.sync.dma_start(gmg[:], gmp[:, gp * 2048:(gp + 1) * 2048])
                return bxg, hgg, gmg

            cur = emit_gp_loads(0)
            nxt = None
            stag = None
            for gp in range(NTT):
                bxg, hgg, gmg = cur
                bxg_v = bxg[:].rearrange("p (m c t b) -> p m c t b",
                                         m=NREC, c=4, t=32, b=16)
                hgg_v = hgg[:].rearrange("p (c t b) -> p c t b",
                                         c=4, t=32, b=16)
                gmg_v = gmg[:].rearrange("p (c t b) -> p c t b",
                                         c=4, t=32, b=16)
                for tl in range(32):
                    t = gp * 32 + tl
                    if tl == 4 and gp + 1 < NTT:
                        nxt = emit_gp_loads(gp + 1)
                    if t % 2 == 0:
                        next(p1gen, None)
                    if t % 8 == 0:
                        stag = p2s.tile([128, 512], dt.float16, tag="stbuf",
                                        name="stbuf")
                    off = (t % 8) * 64
                    psc = [p2ps.tile([128, HW], dt.float32,
                                     tag=f"psc{h}{t % 3}",
                                     name=f"psc{h}{t % 3}")
                           for h in range(2)]
                    for h in range(2):
                        nc.tensor.matmul(
                            psc[h][:], iden[:],
                            bxg_v[:, :, 2 * h:2 * h + 2, tl, :],
                            start=True, stop=False)

                    def amms(hreg, ks, stop_k):
                        for c in (2 * hreg, 2 * hreg + 1):
                            for m in range(NREC):
                                for k in ks:
                                    nc.tensor.matmul(
                                        psc[hreg][:, m * 32 + (c % 2) * 16:
                                                  m * 32 + (c % 2) * 16 + 16],
                                        aw[:, ablk(m, c, k):ablk(m, c, k) + 128],
                                        sth[k // 2][:, (k % 2) * 16:
                                                    (k % 2) * 16 + 16],
                                        start=False, stop=(k == stop_k))
                    amms(0, (0, 1), -1)   # h0 regions, early (sth0-gated)
                    amms(0, (2, 3), 3)    # h0 finishers (sth1-gated)
                    amms(1, (0, 1), -1)   # h1 regions (sth0-gated)
                    amms(1, (2, 3), 3)    # h1 finishers

                    new_sth = [None, None]
                    for h in (0, 1):
                        ft = p2c.tile([128, HW], dt.float16, tag=f"ft{h}")
                        nc.scalar.activation(ft[:], psc[h][:], AF.Tanh)
                        hgs = hgg_v[:, 2 * h:2 * h + 2, tl, :]
                        gms = gmg_v[:, 2 * h:2 * h + 2, tl, :]
                        m2h = p2c.tile([128, 32], dt.float16, tag=f"m2{h}")
                        nc.gpsimd.tensor_tensor(m2h[:], sth[h], gms, ALU.mult)
                        s_new = stag[:, off + h * 32:off + (h + 1) * 32]
                        if NREC == 2 and EQ:
                            w = p2c.tile([128, 32], dt.float16, tag=f"w{h}")
                            nc.vector.tensor_tensor(
                                w[:], ft[:, 0:32], ft[:, 32:64], ALU.add)
                            q = p2c.tile([128, 32], dt.float16, tag=f"q{h}")
                            nc.vector.scalar_tensor_tensor(
                                q[:], w[:], 0.5, hgs, ALU.mult, ALU.mult)
                        elif NREC == 2:
                            mc = p2c.tile([128, 64], dt.float16, tag=f"mc{h}")
                            nc.vector.scalar_tensor_tensor(
                                mc[:, 0:32], ft[:, 0:32], 1.0 - alpha,
                                hgs, ALU.mult, ALU.mult)
                            nc.vector.scalar_tensor_tensor(
                                mc[:, 32:64], ft[:, 32:64], alpha,
                                hgs, ALU.mult, ALU.mult)
                            q = p2c.tile([128, 32], dt.float16, tag=f"q{h}")
                            nc.vector.tensor_tensor(
                                q[:], mc[:, 0:32], mc[:, 32:64], ALU.add)
                        else:
                            q = p2c.tile([128, 32], dt.float16, tag=f"q{h}")
                            nc.vector.scalar_tensor_tensor(
                                q[:], ft[:], 1.0, hgs, ALU.mult, ALU.mult)
                        nc.vector.tensor_tensor(s_new, q[:], m2h[:], ALU.add)
                        new_sth[h] = s_new
                    sth = new_sth
                    if t % 8 == 7:
                        nc.sync.dma_start(
                            stg_d[:, (t - 7) * 64:(t + 1) * 64], stag[:])
                cur = nxt
            for _ in p1gen:
                pass

    nc.compile()
    return nc


def _pack_lhsT_blocks(Wm, kdim, mdim, dtype):
    """Wm: [mdim*128, kdim*128]; returns [128, kdim*mdim*128] with block
    (k, j) at cols (k*mdim+j)*128 equal to Wm[j-chunk, k-chunk].T."""
    out = np.zeros((128, kdim * mdim * 128), dtype=dtype)
    for k in range(kdim):
        for j in range(mdim):
            blk = Wm[j * 128:(j + 1) * 128, k * 128:(k + 1) * 128].T
            out[:, (k * mdim + j) * 128:(k * mdim + j + 1) * 128] = blk
    return np.ascontiguousarray(out)


def kernel(x_seq, s0, A0_w, B0_w, B0_b, A1_w, B1_w, B1_b, gate_w, gate_b,
           alpha, z, _T=None, _trace=False):
    from concourse.bass_utils import run_bass_kernel_spmd

    T = int(_T or T_FULL)
    alpha_f = float(np.asarray(alpha))
    z_i = int(np.asarray(z))

    key = (alpha_f, z_i, T)
    if key not in _cache:
        _cache[key] = _build(alpha_f, z_i, T)
    nc = _cache[key]

    NMAT = 3 if z_i != 0 else 2
    NREC = 2 if z_i != 0 else 1

    x_seq = np.asarray(x_seq, dtype=np.float32)
    s0 = np.asarray(s0, dtype=np.float32)

    # ---- shared (replicated) weight packing ----
    mats = ([np.asarray(B0_w), np.asarray(B1_w), np.asarray(gate_w)]
            if z_i != 0 else [np.asarray(B0_w), np.asarray(gate_w)])
    biases = ([np.asarray(B0_b), np.asarray(B1_b), np.asarray(gate_b)]
              if z_i != 0 else [np.asarray(B0_b), np.asarray(gate_b)])
    # phase-1 lhsT blocks per matrix: (mat, i, c) at col ((mat*2+i)*4+c)*128
    pw = np.concatenate(
        [_pack_lhsT_blocks(Wm.astype(np.float32), 2, 4, np.float32)
         for Wm in mats], axis=1).astype(np.float16)
    pw = np.ascontiguousarray(pw)

    bias = np.zeros((128, 4 * NMAT), np.float32)
    for mi, bvec in enumerate(biases):
        bias[:, mi * 4:(mi + 1) * 4] = bvec.astype(np.float32).reshape(4, 128).T

    recs = [np.asarray(A0_w)] if z_i == 0 else [np.asarray(A0_w), np.asarray(A1_w)]
    # phase-2 lhsT block (m, c, k) at col ((m*4+c)*4+k)*128 = A_m[c128, k128].T
    aw_list = []
    for A in recs:
        Af = A.astype(np.float32)
        blocks = np.zeros((128, 16 * 128), np.float32)
        for c in range(4):
            for k in range(4):
                blocks[:, (c * 4 + k) * 128:(c * 4 + k + 1) * 128] = \
                    Af[c * 128:(c + 1) * 128, k * 128:(k + 1) * 128].T
        aw_list.append(blocks)
    aw = np.ascontiguousarray(np.concatenate(aw_list, axis=1).astype(np.float16))

    IDEN = np.ascontiguousarray(np.eye(128, dtype=np.float16))

    # ---- per-core inputs ----
    in_maps = []
    for cix in range(N_CORES):
        bc = cix * B_LOC
        xc = x_seq[bc:bc + B_LOC, :T]                       # [16, T, 256]
        xT = np.ascontiguousarray(
            xc.transpose(2, 1, 0).reshape(2, 128, T * B_LOC)).astype(np.float16)
        s0c = s0[bc:bc + B_LOC]                             # [16, 512]
        s0T = np.ascontiguousarray(
            s0c.T.reshape(4, 128, B_LOC).transpose(1, 0, 2).reshape(128, 64)
        ).astype(np.float16)
        in_maps.append({
            "xT": xT, "pw": pw, "bias": bias, "aw": aw, "s0T": s0T,
            "iden": IDEN,
        })

    res = run_bass_kernel_spmd(nc, in_maps, list(range(N_CORES)), trace=_trace)
    if _trace:
        kernel._last_res = res

    out = np.empty((B_FULL, T + 1, S_DIM), np.float32)
    for cix in range(N_CORES):
        bc = cix * B_LOC
        stg = np.asarray(res.results[cix]["stg"], dtype=np.float32)  # [128, T*64]
        out[bc:bc + B_LOC, 0] = s0[bc:bc + B_LOC]
        out[bc:bc + B_LOC, 1:] = (
            stg.reshape(128, T, 4, B_LOC).transpose(3, 1, 2, 0)
            .reshape(B_LOC, T, S_DIM))
    return out


# revision 21
# speedup vs baseline: 1.2500x; 1.0050x over previous
"""Trainium2 Bass kernel for the BinaryMechanismSSM problem.

Full inputs in, full outputs out. Batch (128) sharded 8 ways (16 rows/core).

Per core:
  Phase 1: projections bx{0,1} = x @ B{0,1}^T + b (fp16 matmuls, fp32 psum),
           g = sigmoid(x @ G^T + b), 1-g via DVE. Each 512-token tile's
           outputs are collected in wide SBUF staging tiles and stored with
           3 DMAs into group-pair-major DRAM planes (dma_start issue cost
           on the sync engine is ~640ns each, so DMA count is minimized).
  Phase 2: T sequential steps. State held fp16 as slices of an 8-step
           rolling buffer; per step, per half h: one fp16 identity matmul
           injects bx into a [128, NREC*32] PSUM half (start=True), 16 fp16
           A-matmuls accumulate (contraction chunk k reads the previous
           state's half k//2, so matmuls gated on the early half start
           before the late half lands). Tail per half: ACT tanh -> DVE
           pair-add -> (0.5*w)*g -> + (1-g)*s_prev (m2 on GpSimd).
           State is stored to DRAM once per 8 steps.
"""
import numpy as np

B_FULL = 128
T_FULL = 1024
I_DIM = 256
S_DIM = 512
N_CORES = 8
B_LOC = B_FULL // N_CORES  # 16

_cache = {}


def _build(alpha: float, z: int, T: int):
    import concourse.bass as bass
    from concourse import bacc
    import concourse.mybir as mybir
    from concourse.tile import TileContext

    dt = mybir.dt
    AF = mybir.ActivationFunctionType
    ALU = mybir.AluOpType

    TOK = T * B_LOC          # tokens per core
    NTT = TOK // 512         # phase-1 token tiles == group pairs (32 steps)
    NMAT = 3 if z != 0 else 2
    NREC = 2 if z != 0 else 1
    HW = NREC * 32           # psum half width
    WB = NREC * 64           # bx cols per step
    EQ = (z != 0 and abs(alpha - 0.5) < 1e-12)

    nc = bacc.Bacc("TRN2", target_bir_lowering=False, debug=False,
                   num_devices=N_CORES)

    xT_d = nc.declare_dram_parameter("xT", [2, 128, TOK], dt.float16, isOutput=False)
    pw_d = nc.declare_dram_parameter("pw", [128, NMAT * 2 * 4 * 128], dt.float16, isOutput=False)
    bias_d = nc.declare_dram_parameter("bias", [128, 4 * NMAT], dt.float32, isOutput=False)
    aw_d = nc.declare_dram_parameter("aw", [128, NREC * 16 * 128], dt.float16, isOutput=False)
    s0_d = nc.declare_dram_parameter("s0T", [128, 64], dt.float16, isOutput=False)
    iden_d = nc.declare_dram_parameter("iden", [128, 128], dt.float16, isOutput=False)
    stg_d = nc.declare_dram_parameter("stg", [128, T * 64], dt.float16, isOutput=True)

    with TileContext(nc) as tc:
      with tc.tile_pool(name="dram", bufs=1, space="DRAM") as dpool:
        # group-pair-major planes: one contiguous slab per 32 steps
        bxp = dpool.tile([128, NTT * NREC * 4 * 512], dt.float16, tag="bxp",
                         name="bxp")
        hgp = dpool.tile([128, NTT * 4 * 512], dt.float16, tag="hgp",
                         name="hgp")
        gmp = dpool.tile([128, NTT * 4 * 512], dt.float16, tag="gmp",
                         name="gmp")

        # ---------------- pools (both phases coexist; phase 1 is pumped
        # into phase 2's per-step idle gaps) ----------------
        with (
            tc.tile_pool(name="p1w", bufs=1) as p1w,
            tc.tile_pool(name="p1x", bufs=3) as p1x,
            tc.tile_pool(name="p1o", bufs=2) as p1o,
            tc.tile_pool(name="p1ps", bufs=2, space="PSUM") as p1ps,
            tc.tile_pool(name="p2w", bufs=1) as p2w,
            tc.tile_pool(name="p2g", bufs=2) as p2g,
            tc.tile_pool(name="p2s", bufs=3) as p2s,
            tc.tile_pool(name="p2c", bufs=6) as p2c,
            tc.tile_pool(name="p2ps", bufs=1, space="PSUM") as p2ps,
        ):
            pw = p1w.tile([128, NMAT * 2 * 4 * 128], dt.float16)
            nc.sync.dma_start(pw[:], pw_d[:])
            bias = p1w.tile([128, 4 * NMAT], dt.float32)
            nc.sync.dma_start(bias[:], bias_d[:])

            def p1_units():
                """Phase-1 work as small bundles; yields between bundles so
                they can be sprinkled across phase-2 steps."""
                W1 = NREC * 4 * 512
                for tt in range(NTT):
                    xt = p1x.tile([128, 2 * 512], dt.float16, tag="xt",
                                  name="xt")
                    for i in range(2):
                        nc.sync.dma_start(xt[:, i * 512:(i + 1) * 512],
                                          xT_d[i, :, tt * 512:(tt + 1) * 512])
                    obx = p1o.tile([128, NREC * 4 * 512], dt.float16,
                                   tag="obx", name="obx")
                    ohg = p1o.tile([128, 4 * 512], dt.float16, tag="ohg",
                                   name="ohg")
                    ogm = p1o.tile([128, 4 * 512], dt.float16, tag="ogm",
                                   name="ogm")
                    for mat in range(NMAT):
                        for c in range(4):
                            ps = p1ps.tile([128, 512], dt.float32, tag="pps",
                                           name="pps")
                            for i in range(2):
                                blk = ((mat * 2 + i) * 4 + c) * 128
                                nc.tensor.matmul(
                                    ps[:], pw[:, blk:blk + 128],
                                    xt[:, i * 512:(i + 1) * 512],
                                    start=(i == 0), stop=(i == 1))
                            bj = bias[:, mat * 4 + c:mat * 4 + c + 1]
                            if mat < NREC:
                                o = obx[:, (mat * 4 + c) * 512:
                                        (mat * 4 + c + 1) * 512]
                                # halves, alternating DVE/ACT: small quanta
                                # that don't block the recurrence chain
                                for ih in range(2):
                                    osl = o[:, ih * 256:(ih + 1) * 256]
                                    psl = ps[:, ih * 256:(ih + 1) * 256]
                                    if (mat * 4 + c) % 2 == 0:
                                        nc.vector.tensor_scalar(
                                            osl, psl, bj, None, ALU.add)
                                    else:
                                        nc.scalar.activation(
                                            osl, psl, AF.Identity,
                                            bias=bj, scale=1.0)
                            else:
                                sg = ohg[:, c * 512:(c + 1) * 512]
                                for ih in range(2):
                                    nc.scalar.activation(
                                        sg[:, ih * 256:(ih + 1) * 256],
                                        ps[:, ih * 256:(ih + 1) * 256],
                                        AF.Sigmoid, bias=bj, scale=1.0)
                                nc.gpsimd.tensor_scalar(
                                    ogm[:, c * 512:(c + 1) * 512],
                                    sg, -1.0, 1.0, ALU.mult, ALU.add)
                            yield
                    nc.sync.dma_start(bxp[:, tt * W1:(tt + 1) * W1], obx[:])
                    nc.sync.dma_start(hgp[:, tt * 2048:(tt + 1) * 2048],
                                      ohg[:])
                    nc.sync.dma_start(gmp[:, tt * 2048:(tt + 1) * 2048],
                                      ogm[:])
                    yield

            UNITS_PER_TILE = 4 * NMAT + 1
            LEAD_TILES = min(4, NTT)
            p1gen = p1_units()
            for _ in range(LEAD_TILES * UNITS_PER_TILE):
                next(p1gen, None)
            aw = p2w.tile([128, NREC * 16 * 128], dt.float16)
            nc.sync.dma_start(aw[:], aw_d[:])
            iden = p2w.tile([128, 128], dt.float16)
            nc.sync.dma_start(iden[:], iden_d[:])

            s_init = p2w.tile([128, 64], dt.float16)
            nc.sync.dma_start(s_init[:], s0_d[:])
            sth = [s_init[:, 0:32], s_init[:, 32:64]]

            def ablk(m, c, k):
                return ((m * 4 + c) * 4 + k) * 128

            def emit_gp_loads(gp):
                W1 = NREC * 4 * 512
                bxg = p2g.tile([128, W1], dt.float16, tag="bxg", name="bxg")
                nc.sync.dma_start(bxg[:], bxp[:, gp * W1:(gp + 1) * W1])
                hgg = p2g.tile([128, 2048], dt.float16, tag="hgg", name="hgg")
                nc.sync.dma_start(hgg[:], hgp[:, gp * 2048:(gp + 1) * 2048])
                gmg = p2g.tile([128, 2048], dt.float16, tag="gmg", name="gmg")
                nc.sync.dma_start(gmg[:], gmp[:, gp * 2048:(gp + 1) * 2048])
                hgt = p2g.tile([128, 2048], dt.float16, tag="hgt", name="hgt")
                return bxg, hgg, gmg, hgt

            def repack_chunk(tiles, k):
                # (c, t, b) -> (t, c, b): per-step gate slices contiguous
                _, hgg, _, hgt = tiles
                src = hgg[:].rearrange("p (c t b) -> p t c b",
                                       c=4, t=32, b=16)
                dst = hgt[:].rearrange("p (t c b) -> p t c b",
                                       t=32, c=4, b=16)
                nc.vector.tensor_scalar(
                    dst[:, 8 * k:8 * k + 8], src[:, 8 * k:8 * k + 8],
                    0.0, None, ALU.add)

            cur = emit_gp_loads(0)
            for k in range(4):
                repack_chunk(cur, k)
            nxt = None
            stag = None
            for gp in range(NTT):
                bxg, hgg, gmg, hgt = cur
                bxg_v = bxg[:].rearrange("p (m c t b) -> p m c t b",
                                         m=NREC, c=4, t=32, b=16)
                gmg_v = gmg[:].rearrange("p (c t b) -> p c t b",
                                         c=4, t=32, b=16)
                for tl in range(32):
                    t = gp * 32 + tl
                    if tl == 4 and gp + 1 < NTT:
                        nxt = emit_gp_loads(gp + 1)
                    if tl in (6, 8, 10, 12) and nxt is not None and \
                            gp + 1 < NTT:
                        repack_chunk(nxt, (tl - 6) // 2)
                    if t % 2 == 0:
                        next(p1gen, None)
                    if t % 8 == 0:
                        stag = p2s.tile([128, 512], dt.float16, tag="stbuf",
                                        name="stbuf")
                    off = (t % 8) * 64
                    psc = [p2ps.tile([128, HW], dt.float32,
                                     tag=f"psc{h}{t % 3}",
                                     name=f"psc{h}{t % 3}")
                           for h in range(2)]
                    for h in range(2):
                        nc.tensor.matmul(
                            psc[h][:], iden[:],
                            bxg_v[:, :, 2 * h:2 * h + 2, tl, :],
                            start=True, stop=False)

                    def amms(hreg, ks, stop_k):
                        for c in (2 * hreg, 2 * hreg + 1):
                            for m in range(NREC):
                                for k in ks:
                                    nc.tensor.matmul(
                                        psc[hreg][:, m * 32 + (c % 2) * 16:
                                                  m * 32 + (c % 2) * 16 + 16],
                                        aw[:, ablk(m, c, k):ablk(m, c, k) + 128],
                                        sth[k // 2][:, (k % 2) * 16:
                                                    (k % 2) * 16 + 16],
                                        start=False, stop=(k == stop_k))
                    amms(0, (0, 1), -1)   # h0 regions, early (sth0-gated)
                    amms(0, (2, 3), 3)    # h0 finishers (sth1-gated)
                    amms(1, (0, 1), -1)   # h1 regions (sth0-gated)
                    amms(1, (2, 3), 3)    # h1 finishers

                    new_sth = [None, None]
                    for h in (0, 1):
                        ft = p2c.tile([128, HW], dt.float16, tag=f"ft{h}")
                        nc.scalar.activation(ft[:], psc[h][:], AF.Tanh)
                        hgs = hgt[:, tl * 64 + h * 32:tl * 64 + h * 32 + 32]
                        gms = gmg_v[:, 2 * h:2 * h + 2, tl, :]
                        m2h = p2c.tile([128, 32], dt.float16, tag=f"m2{h}")
                        nc.gpsimd.tensor_tensor(m2h[:], sth[h], gms, ALU.mult)
                        s_new = stag[:, off + h * 32:off + (h + 1) * 32]
                        if NREC == 2 and EQ:
                            w = p2c.tile([128, 32], dt.float16, tag=f"w{h}")
                            nc.vector.tensor_tensor(
                                w[:], ft[:, 0:32], ft[:, 32:64], ALU.add)
                            q = p2c.tile([128, 32], dt.float16, tag=f"q{h}")
                            nc.vector.scalar_tensor_tensor(
                                q[:], w[:], 0.5, hgs, ALU.mult, ALU.mult)
                        elif NREC == 2:
                            mc = p2c.tile([128, 64], dt.float16, tag=f"mc{h}")
                            nc.vector.scalar_tensor_tensor(
                                mc[:, 0:32], ft[:, 0:32], 1.0 - alpha,
                                hgs, ALU.mult, ALU.mult)
                            nc.vector.scalar_tensor_tensor(
                                mc[:, 32:64], ft[:, 32:64], alpha,
                                hgs, ALU.mult, ALU.mult)
                            q = p2c.tile([128, 32], dt.float16, tag=f"q{h}")
                            nc.vector.tensor_tensor(
                                q[:], mc[:, 0:32], mc[:, 32:64], ALU.add)
                        else:
                            q = p2c.tile([128, 32], dt.float16, tag=f"q{h}")
                            nc.vector.scalar_tensor_tensor(
                                q[:], ft[:], 1.0, hgs, ALU.mult, ALU.mult)
                        nc.vector.tensor_tensor(s_new, q[:], m2h[:], ALU.add)
                        new_sth[h] = s_new
                    sth = new_sth
                    if t % 8 == 7:
                        nc.sync.dma_start(
                            stg_d[:, (t - 7) * 64:(t + 1) * 64], stag[:])
                cur = nxt
            for _ in p1gen:
                pass

    nc.compile()
    return nc


def _pack_lhsT_blocks(Wm, kdim, mdim, dtype):
    """Wm: [mdim*128, kdim*128]; returns [128, kdim*mdim*128] with block
    (k, j) at cols (k*mdim+j)*128 equal to Wm[j-chunk, k-chunk].T."""
    out = np.zeros((128, kdim * mdim * 128), dtype=dtype)
    for k in range(kdim):
        for j in range(mdim):
            blk = Wm[j * 128:(j + 1) * 128, k * 128:(k + 1) * 128].T
            out[:, (k * mdim + j) * 128:(k * mdim + j + 1) * 128] = blk
    return np.ascontiguousarray(out)


def kernel(x_seq, s0, A0_w, B0_w, B0_b, A1_w, B1_w, B1_b, gate_w, gate_b,
           alpha, z, _T=None, _trace=False):
    from concourse.bass_utils import run_bass_kernel_spmd

    T = int(_T or T_FULL)
    alpha_f = float(np.asarray(alpha))
    z_i = int(np.asarray(z))

    key = (alpha_f, z_i, T)
    if key not in _cache:
        _cache[key] = _build(alpha_f, z_i, T)
    nc = _cache[key]

    NMAT = 3 if z_i != 0 else 2
    NREC = 2 if z_i != 0 else 1

    x_seq = np.asarray(x_seq, dtype=np.float32)
    s0 = np.asarray(s0, dtype=np.float32)

    # ---- shared (replicated) weight packing ----
    mats = ([np.asarray(B0_w), np.asarray(B1_w), np.asarray(gate_w)]
            if z_i != 0 else [np.asarray(B0_w), np.asarray(gate_w)])
    biases = ([np.asarray(B0_b), np.asarray(B1_b), np.asarray(gate_b)]
              if z_i != 0 else [np.asarray(B0_b), np.asarray(gate_b)])
    # phase-1 lhsT blocks per matrix: (mat, i, c) at col ((mat*2+i)*4+c)*128
    pw = np.concatenate(
        [_pack_lhsT_blocks(Wm.astype(np.float32), 2, 4, np.float32)
         for Wm in mats], axis=1).astype(np.float16)
    pw = np.ascontiguousarray(pw)

    bias = np.zeros((128, 4 * NMAT), np.float32)
    for mi, bvec in enumerate(biases):
        bias[:, mi * 4:(mi + 1) * 4] = bvec.astype(np.float32).reshape(4, 128).T

    recs = [np.asarray(A0_w)] if z_i == 0 else [np.asarray(A0_w), np.asarray(A1_w)]
    # phase-2 lhsT block (m, c, k) at col ((m*4+c)*4+k)*128 = A_m[c128, k128].T
    aw_list = []
    for A in recs:
        Af = A.astype(np.float32)
        blocks = np.zeros((128, 16 * 128), np.float32)
        for c in range(4):
            for k in range(4):
                blocks[:, (c * 4 + k) * 128:(c * 4 + k + 1) * 128] = \
                    Af[c * 128:(c + 1) * 128, k * 128:(k + 1) * 128].T
        aw_list.append(blocks)
    aw = np.ascontiguousarray(np.concatenate(aw_list, axis=1).astype(np.float16))

    IDEN = np.ascontiguousarray(np.eye(128, dtype=np.float16))

    # ---- per-core inputs ----
    in_maps = []
    for cix in range(N_CORES):
        bc = cix * B_LOC
        xc = x_seq[bc:bc + B_LOC, :T]                       # [16, T, 256]
        xT = np.ascontiguousarray(
            xc.transpose(2, 1, 0).reshape(2, 128, T * B_LOC)).astype(np.float16)
        s0c = s0[bc:bc + B_LOC]                             # [16, 512]
        s0T = np.ascontiguousarray(
            s0c.T.reshape(4, 128, B_LOC).transpose(1, 0, 2).reshape(128, 64)
        ).astype(np.float16)
        in_maps.append({
            "xT": xT, "pw": pw, "bias": bias, "aw": aw, "s0T": s0T,
            "iden": IDEN,
        })

    res = run_bass_kernel_spmd(nc, in_maps, list(range(N_CORES)), trace=_trace)
    if _trace:
        kernel._last_res = res

    out = np.empty((B_FULL, T + 1, S_DIM), np.float32)
    for cix in range(N_CORES):
        bc = cix * B_LOC
        stg = np.asarray(res.results[cix]["stg"], dtype=np.float32)  # [128, T*64]
        out[bc:bc + B_LOC, 0] = s0[bc:bc + B_LOC]
        out[bc:bc + B_LOC, 1:] = (
            stg.reshape(128, T, 4, B_LOC).transpose(3, 1, 2, 0)
            .reshape(B_LOC, T, S_DIM))
    return out
